# revision 1
# baseline (speedup 1.0000x reference)
import numpy as np

import concourse.bacc as bacc
import concourse.tile as tile
import concourse.mybir as mybir
from concourse.bass_utils import run_bass_kernel_spmd

F32 = mybir.dt.float32
F32R = mybir.dt.float32r

B = 4
N = 2048
PD = 512
CD = 128
ID = 512
OUT_D = 512
H_PER_CORE = 4
DH = 64
SCALE = 0.125          # dim_head ** -0.5
NT = 4                 # n chunks of 512
NP = 4                 # pd chunks of 128
NKT = 16               # key tiles of 128
VW = 65 * H_PER_CORE   # vtile columns per key tile: 4 x (64 V | 1 one)

_NC = None
LAST_EXEC_NS = None


def _build():
    nc = bacc.Bacc("TRN2", target_bir_lowering=False, debug=False, num_devices=8)
    XT = nc.declare_dram_parameter("XT", [PD, N], F32, isOutput=False)
    CT = nc.declare_dram_parameter("CT", [CD, N], F32, isOutput=False)
    Wqp = nc.declare_dram_parameter("Wqp", [128, 1024], F32, isOutput=False)
    Wkp = nc.declare_dram_parameter("Wkp", [128, 1024], F32, isOutput=False)
    Wv = nc.declare_dram_parameter("Wv", [128, 1024], F32, isOutput=False)
    Wqc = nc.declare_dram_parameter("Wqc", [CD, 256], F32, isOutput=False)
    Wkc = nc.declare_dram_parameter("Wkc", [CD, 256], F32, isOutput=False)
    Wo = nc.declare_dram_parameter("Wo", [128, 1024], F32, isOutput=False)
    Y = nc.declare_dram_parameter("Y", [N, OUT_D], F32, isOutput=True)

    MIN = mybir.AluOpType.min
    MAX = mybir.AluOpType.max
    MULT = mybir.AluOpType.mult
    ADD = mybir.AluOpType.add
    EXP = mybir.ActivationFunctionType.Exp

    with tile.TileContext(nc) as tc, \
         tc.tile_pool(name="persist", bufs=1) as pp, \
         tc.tile_pool(name="work", bufs=2) as wk, \
         tc.tile_pool(name="pb_s", bufs=2, space="PSUM") as psp, \
         tc.tile_pool(name="pb_o", bufs=2, space="PSUM") as pop, \
         tc.tile_pool(name="pc_y", bufs=2, space="PSUM") as pyp:
        xt = [pp.tile([128, N], F32R, name=f"xt{p}", tag=f"xt{p}") for p in range(NP)]
        ct = pp.tile([128, N], F32R, name="ct", tag="ct")
        wqp = pp.tile([128, 1024], F32R, name="wqp", tag="wqp")
        wkp = pp.tile([128, 1024], F32R, name="wkp", tag="wkp")
        wv = pp.tile([128, 1024], F32R, name="wv", tag="wv")
        wqc = pp.tile([128, 256], F32R, name="wqc", tag="wqc")
        wkc = pp.tile([128, 256], F32R, name="wkc", tag="wkc")
        wo = pp.tile([128, 1024], F32R, name="wo", tag="wo")
        qcat = [pp.tile([128, N], F32R, name=f"qcat{h}", tag=f"qcat{h}")
                for h in range(H_PER_CORE)]
        kcat = [pp.tile([128, N], F32R, name=f"kcat{h}", tag=f"kcat{h}")
                for h in range(H_PER_CORE)]
        vtile = pp.tile([128, NKT * VW], F32R, name="vtile", tag="vtile")
        ocat = [pp.tile([128, N], F32R, name=f"ocat{j}", tag=f"ocat{j}")
                for j in range(2)]

        # Input DMAs split across two queues (SP + Pool doorbell) so the
        # critical prologue tensors land sooner.
        nc.sync.dma_start(wkp[:], Wkp[:].bitcast(F32R))
        nc.gpsimd.dma_start(wkc[:], Wkc[:].bitcast(F32R))
        nc.sync.dma_start(wqp[:], Wqp[:].bitcast(F32R))
        nc.gpsimd.dma_start(wqc[:], Wqc[:].bitcast(F32R))
        nc.sync.dma_start(xt[0][:], XT[0:128, :].bitcast(F32R))
        nc.gpsimd.dma_start(xt[2][:], XT[256:384, :].bitcast(F32R))
        nc.sync.dma_start(xt[1][:], XT[128:256, :].bitcast(F32R))
        nc.gpsimd.dma_start(xt[3][:], XT[384:512, :].bitcast(F32R))
        nc.sync.dma_start(ct[:], CT[:].bitcast(F32R))
        nc.gpsimd.dma_start(wv[:], Wv[:].bitcast(F32R))
        nc.sync.dma_start(wo[:], Wo[:].bitcast(F32R))

        # Everything (QKV projections, attention, output projection) runs in
        # one flat software pipeline over 128 kt-pairs (4 qi x 4 h x 8 j).
        # Steady state per pair g: ACT(exp) of pair g, AV matmuls of pair g,
        # dots matmuls of pair g+2 (so ACT never starves behind the in-order
        # PE queue). K/V/Q projections, norm chains, and the output
        # projection are spread into PE slack with emission-order deadlines.
        NPAIR = NT * H_PER_CORE * (NKT // 2)
        if True:
            ps_tiles = {}
            po_tiles = {}
            qstate = {}

            def proj_tg(t, g, wpix, wcrd, dest):
                # qk projection for column block t, head-group g (heads
                # 2g, 2g+1): pixel part into rows 0:64, coord into 64:128.
                ps = pyp.tile([128, 512], F32, name="py", tag="py")
                for c in range(NP):
                    nc.tensor.matmul(
                        ps[:], wpix[:, c * 256 + g * 128:c * 256 + (g + 1) * 128],
                        xt[c][:, t * 512:(t + 1) * 512],
                        start=(c == 0), stop=(c == NP - 1))
                for jj in range(2):
                    nc.vector.tensor_scalar(
                        dest[2 * g + jj][0:64, t * 512:(t + 1) * 512],
                        ps[jj * 64:(jj + 1) * 64, :], 5.0, -5.0, op0=MIN, op1=MAX)
                ps2 = pyp.tile([128, 512], F32, name="py", tag="py")
                nc.tensor.matmul(ps2[:, 0:512], wcrd[:, g * 128:(g + 1) * 128],
                                 ct[:, t * 512:(t + 1) * 512], start=True, stop=True)
                for jj in range(2):
                    nc.vector.tensor_scalar(
                        dest[2 * g + jj][64:128, t * 512:(t + 1) * 512],
                        ps2[jj * 64:(jj + 1) * 64, :], 5.0, -5.0, op0=MIN, op1=MAX)

            def v_kt(kt):
                t, i = divmod(kt, 4)
                pv = pyp.tile([128, 512], F32, name="py", tag="py")
                for p in range(NP):
                    nc.tensor.matmul(
                        pv[:, 0:256],
                        xt[p][:, t * 512 + i * 128:t * 512 + (i + 1) * 128],
                        wv[:, p * 256:(p + 1) * 256],
                        start=(p == 0), stop=(p == NP - 1))
                for h in range(H_PER_CORE):
                    nc.vector.tensor_copy(
                        vtile[:, kt * VW + h * 65:kt * VW + h * 65 + 64],
                        pv[:, h * 64:(h + 1) * 64])

            def loc(g):
                qi, r = divmod(g, 32)
                h, j = divmod(r, 8)
                return qi, h, j

            def emit_dots(g):
                qi, h, j = loc(g)
                ps = psp.tile([128, 1024], F32, name="ps", tag="ps")
                ps_tiles[g] = ps
                k0, k1 = 2 * j, 2 * j + 1
                nc.tensor.matmul(
                    ps[:, 0:512], kcat[h][:, k0 * 128:(k0 + 1) * 128],
                    qcat[h][:, qi * 512:(qi + 1) * 512], start=True, stop=True)
                nc.tensor.matmul(
                    ps[:, 512:1024], kcat[h][:, k1 * 128:(k1 + 1) * 128],
                    qcat[h][:, qi * 512:(qi + 1) * 512], start=True, stop=True)

            def emit_av(g, pe):
                qi, h, j = loc(g)
                if j == 0:
                    po_tiles[(qi, h)] = pop.tile([65, 512], F32, name="po", tag="po")
                po = po_tiles[(qi, h)]
                k0, k1 = 2 * j, 2 * j + 1
                nc.tensor.matmul(
                    po[:], vtile[:, k0 * VW + h * 65:k0 * VW + h * 65 + 65],
                    pe[:, 0:512], start=(j == 0), stop=False)
                nc.tensor.matmul(
                    po[:], vtile[:, k1 * VW + h * 65:k1 * VW + h * 65 + 65],
                    pe[:, 512:1024], start=False, stop=(j == NKT // 2 - 1))

            def emit_norm(qi, h):
                po = po_tiles[(qi, h)]
                r = wk.tile([1, 512], F32, name="r", tag="r")
                rb = wk.tile([64, 512], F32, name="rb", tag="rb")
                nc.vector.reciprocal(r[:], po[64:65, :])
                nc.gpsimd.partition_broadcast(rb[:], r[:])
                oj, rr = h // 2, (h % 2) * 64
                nc.vector.tensor_tensor(
                    ocat[oj][rr:rr + 64, qi * 512:(qi + 1) * 512],
                    po[0:64, :], rb[:], op=MULT)

            def emit_phasec(qi, i):
                n0 = qi * 512 + i * 128
                py = pyp.tile([128, 512], F32, name="py", tag="py")
                nc.tensor.matmul(py[:], ocat[0][:, n0:n0 + 128], wo[:, 0:512],
                                 start=True, stop=False)
                nc.tensor.matmul(py[:], ocat[1][:, n0:n0 + 128], wo[:, 512:1024],
                                 start=False, stop=True)
                st = wk.tile([128, 512], F32, name="st", tag="st")
                nc.vector.tensor_copy(st[:], py[:])
                nc.sync.dma_start(Y[n0:n0 + 128, :], st[:])

            def emit_qproj_piece(t, p):
                # p 0..7: pixel-part matmul (group p//4, x chunk p%4);
                # p 8..9: coord-part matmul for group p-8. Clamps follow the
                # final accumulating matmul (they run on DVE, not PE).
                if p < 8:
                    g, c = divmod(p, 4)
                    if c == 0:
                        qstate[(t, g)] = pyp.tile([128, 512], F32,
                                                  name="py", tag="py")
                    ps = qstate[(t, g)]
                    nc.tensor.matmul(
                        ps[:], wqp[:, c * 256 + g * 128:c * 256 + (g + 1) * 128],
                        xt[c][:, t * 512:(t + 1) * 512],
                        start=(c == 0), stop=(c == 3))
                    if c == 3:
                        for jj in range(2):
                            nc.vector.tensor_scalar(
                                qcat[2 * g + jj][0:64, t * 512:(t + 1) * 512],
                                ps[jj * 64:(jj + 1) * 64, :], 5.0, -5.0,
                                op0=MIN, op1=MAX)
                else:
                    g = p - 8
                    ps = pyp.tile([128, 512], F32, name="py", tag="py")
                    nc.tensor.matmul(ps[:], wqc[:, g * 128:(g + 1) * 128],
                                     ct[:, t * 512:(t + 1) * 512],
                                     start=True, stop=True)
                    for jj in range(2):
                        nc.vector.tensor_scalar(
                            qcat[2 * g + jj][64:128, t * 512:(t + 1) * 512],
                            ps[jj * 64:(jj + 1) * 64, :], 5.0, -5.0,
                            op0=MIN, op1=MAX)

            def ones_cols(c0, c1, w):
                # vtile ones-columns: in*0 + 1, seeded from xt[0]
                nc.vector.tensor_scalar(vtile[:, c0:c1], xt[0][:, 0:w], 0.0, 1.0,
                                        op0=MULT, op1=ADD)

            # Prologue: minimal K/Q prefix so exp can start ASAP, then the
            # V tiles and K columns needed by the first few pipeline pairs.
            proj_tg(0, 0, wkp, wkc, kcat)
            proj_tg(0, 0, wqp, wqc, qcat)
            emit_dots(0)
            emit_dots(1)
            ones_cols(0, 2048, 2048)
            for kt in range(4):
                v_kt(kt)
            proj_tg(1, 0, wkp, wkc, kcat)
            ones_cols(2048, 4096, 2048)
            ones_cols(4096, NKT * VW, 64)
            for kt in range(4, 8):
                v_kt(kt)

            # Filler pieces for the qi=0 block, keyed by pipeline position;
            # each fits the ~1us PE slack without starving ACT, and lands
            # before its first consumer's emission point.
            fill0 = {
                0: [lambda: proj_tg(2, 0, wkp, wkc, kcat), lambda: v_kt(8)],
                1: [lambda: v_kt(9), lambda: v_kt(10)],
                2: [lambda: proj_tg(3, 0, wkp, wkc, kcat), lambda: v_kt(11)],
                3: [lambda: v_kt(12), lambda: v_kt(13)],
                4: [lambda: v_kt(14), lambda: v_kt(15)],
                5: [lambda: proj_tg(0, 1, wkp, wkc, kcat)],
                6: [lambda: proj_tg(1, 1, wkp, wkc, kcat)],
                7: [lambda: proj_tg(2, 1, wkp, wkc, kcat)],
                8: [lambda: proj_tg(3, 1, wkp, wkc, kcat)],
                9: [lambda: proj_tg(0, 1, wqp, wqc, qcat)],
            }

            for g in range(NPAIR):
                qi, h, j = loc(g)
                pe = wk.tile([128, 1024], F32R, name="pe", tag="pe", bufs=3)
                nc.scalar.activation(pe[:], ps_tiles[g][:], EXP, scale=SCALE)
                emit_av(g, pe)
                if g + 2 < NPAIR:
                    emit_dots(g + 2)
                r32 = g % 32
                if qi == 0 and r32 in fill0:
                    for f in fill0[r32]:
                        f()
                if qi >= 1 and r32 in (2, 6, 10, 14):
                    emit_phasec(qi - 1, (r32 - 2) // 4)
                if qi + 1 < NT and 16 <= r32 <= 25:
                    emit_qproj_piece(qi + 1, r32 - 16)
                if j == NKT // 2 - 1:
                    emit_norm(qi, h)
            for i in range(4):
                emit_phasec(NT - 1, i)
    nc.compile()
    return nc


def _get_nc():
    global _NC
    if _NC is None:
        _NC = _build()
    return _NC


def _pack(w, nblk, blk):
    w = np.asarray(w, dtype=np.float32)
    return np.ascontiguousarray(
        w.reshape(nblk, 128, blk).transpose(1, 0, 2).reshape(128, nblk * blk))


def kernel(pixels, coords, mask, W_qkv, W_qkc, W_out, b_out):
    global LAST_EXEC_NS
    pixels = np.asarray(pixels, dtype=np.float32)
    coords = np.asarray(coords, dtype=np.float32)
    W_qkv = np.asarray(W_qkv, dtype=np.float32)
    W_qkc = np.asarray(W_qkc, dtype=np.float32)
    W_out = np.asarray(W_out, dtype=np.float32)
    b_out = np.asarray(b_out, dtype=np.float32)

    nc = _get_nc()

    XT = [np.ascontiguousarray(pixels[b].T) for b in range(B)]
    CT = [np.ascontiguousarray(coords[b].T) for b in range(B)]

    in_maps = []
    for c in range(8):
        b = c // 2
        h0 = (c % 2) * H_PER_CORE * DH     # 0 or 256: col offset within split
        in_maps.append({
            "XT": XT[b],
            "CT": CT[b],
            "Wqp": _pack(W_qkv[:, h0:h0 + 256], 4, 256),
            "Wkp": _pack(W_qkv[:, ID + h0:ID + h0 + 256], 4, 256),
            "Wv": _pack(W_qkv[:, 2 * ID + h0:2 * ID + h0 + 256], 4, 256),
            "Wqc": np.ascontiguousarray(W_qkc[:, h0:h0 + 256]),
            "Wkc": np.ascontiguousarray(W_qkc[:, ID + h0:ID + h0 + 256]),
            "Wo": _pack(W_out[h0:h0 + 256, :], 2, 512),
        })

    res = run_bass_kernel_spmd(nc, in_maps, core_ids=list(range(8)))
    LAST_EXEC_NS = getattr(res, "exec_time_ns", None)

    out = np.empty((B, N, OUT_D), np.float32)
    for b in range(B):
        out[b] = res.results[2 * b]["Y"] + res.results[2 * b + 1]["Y"]
    out += b_out[None, None, :]
    return tuple(np.split(out, [1024], axis=1))



# revision 5
# speedup vs baseline: 1.2581x; 1.2581x over previous
import numpy as np
import ml_dtypes

import concourse.bacc as bacc
import concourse.tile as tile
import concourse.mybir as mybir
from concourse.bass_utils import run_bass_kernel_spmd

F32 = mybir.dt.float32
F32R = mybir.dt.float32r
BF16 = mybir.dt.bfloat16

B = 4
N = 2048
PD = 512
CD = 128
ID = 512
OUT_D = 512
H_PER_CORE = 4
DH = 64
SCALE = 0.125          # dim_head ** -0.5
NT = 4                 # n chunks of 512 (query blocks)
NP = 4                 # pd chunks of 128
NKT = 16               # key tiles of 128
VW = 65 * H_PER_CORE   # vtile columns per key tile: 4 x (64 V | 1 one)

_NC = None
LAST_EXEC_NS = None


def _build():
    nc = bacc.Bacc("TRN2", target_bir_lowering=False, debug=False, num_devices=8)
    XT = nc.declare_dram_parameter("XT", [PD, N], BF16, isOutput=False)
    CT = nc.declare_dram_parameter("CT", [CD, N], BF16, isOutput=False)
    W1 = nc.declare_dram_parameter("W1", [128, 2560], BF16, isOutput=False)
    W2 = nc.declare_dram_parameter("W2", [128, 2176], BF16, isOutput=False)
    Y = nc.declare_dram_parameter("Y", [N, OUT_D], BF16, isOutput=True)

    MIN = mybir.AluOpType.min
    MAX = mybir.AluOpType.max
    MULT = mybir.AluOpType.mult
    EXP = mybir.ActivationFunctionType.Exp

    with tile.TileContext(nc) as tc, \
         tc.tile_pool(name="persist", bufs=1) as pp, \
         tc.tile_pool(name="work", bufs=2) as wk, \
         tc.tile_pool(name="pb_s", bufs=2, space="PSUM") as psp, \
         tc.tile_pool(name="pb_o", bufs=2, space="PSUM") as pop, \
         tc.tile_pool(name="pb_y", bufs=2, space="PSUM") as pyp:
        x4 = pp.tile([128, NP * N], BF16, name="x4", tag="x4")
        xt = [x4[:, p * N:(p + 1) * N] for p in range(NP)]
        ct = pp.tile([128, N], BF16, name="ct", tag="ct")
        w1 = pp.tile([128, 2560], BF16, name="w1", tag="w1")
        wkp = w1[:, 0:1024]
        wkc = w1[:, 1024:1280]
        wqp = w1[:, 1280:2304]
        wqc = w1[:, 2304:2560]
        w2 = pp.tile([128, 2176], BF16, name="w2", tag="w2")
        wv = w2[:, 0:1024]
        idt = w2[:, 1024:1152]
        wob = w2[:, 1152:2176]
        qcat = [pp.tile([128, N], F32R, name=f"qcat{h}", tag=f"qcat{h}")
                for h in range(H_PER_CORE)]
        kcat = [pp.tile([128, N], F32R, name=f"kcat{h}", tag=f"kcat{h}")
                for h in range(H_PER_CORE)]
        vtile = pp.tile([128, NKT * VW], BF16, name="vtile", tag="vtile")
        ocat = [pp.tile([128, N], BF16, name=f"ocat{j}", tag=f"ocat{j}")
                for j in range(2)]
        zwarm = pp.tile([128, 512], BF16, name="zwarm", tag="zwarm")

        # PE warmup is enqueued before any DMA so the p-state ramp starts
        # immediately: the zwarm memset goes on Pool ahead of its DMA issues.
        nc.gpsimd.memset(zwarm[:], 0.0)
        pwarm = pyp.tile([128, 512], F32, name="py", tag="py")
        for _ in range(7):
            nc.tensor.matmul(pwarm[:], zwarm[:, 0:128], zwarm[:],
                             start=True, stop=True)

        # Input DMAs over three queues (SP / Pool / ACT doorbells), ordered
        # so the K/Q projection prerequisites for the t=0 block land first.
        # The ACT-queue DMAs are configured before the first Activation and
        # only carry tensors that aren't needed until well into the run.
        nc.gpsimd.memset(vtile[:], 1.0)
        x4v = x4[:].rearrange("p (c n) -> p c n", c=NP, n=N)
        xtv = XT[:].rearrange("(c r) n -> r c n", c=NP, r=128)
        nc.sync.dma_start(w1[:], W1[:])
        nc.scalar.dma_start(ct[:, 0:512], CT[:, 0:512])
        nc.sync.dma_start(x4v[:, :, 0:512], xtv[:, :, 0:512])
        nc.scalar.dma_start(w2[:], W2[:])
        nc.sync.dma_start(x4v[:, :, 512:1024], xtv[:, :, 512:1024])
        nc.scalar.dma_start(ct[:, 512:2048], CT[:, 512:2048])
        nc.sync.dma_start(x4v[:, :, 1024:1536], xtv[:, :, 1024:1536])
        nc.sync.dma_start(x4v[:, :, 1536:2048], xtv[:, :, 1536:2048])

        ps_tiles = {}
        po_tiles = {}
        poc_tiles = {}

        def proj_pix(t, g, wpix, dest, pool=None):
            # pixel half of the q/k projection for block t, head group g:
            # clamped rows 0:64 of heads 2g, 2g+1. Clamps on DVE.
            if pool is None:
                ps = pyp.tile([128, 512], F32, name="py", tag="py")
            else:
                ps = pool.tile([128, 512], F32, name="ps", tag="ps")
            for c in range(NP):
                nc.tensor.matmul(
                    ps[:], wpix[:, c * 256 + g * 128:c * 256 + (g + 1) * 128],
                    xt[c][:, t * 512:(t + 1) * 512],
                    start=(c == 0), stop=(c == NP - 1))
            for jj in range(2):
                nc.vector.tensor_scalar(
                    dest[2 * g + jj][0:64, t * 512:(t + 1) * 512],
                    ps[jj * 64:(jj + 1) * 64, :], 5.0, -5.0, op0=MIN, op1=MAX)

        def proj_crd(t, g, wcrd, dest, eng=None):
            # coord half: clamped rows 64:128. Clamps on Pool by default.
            ps2 = pyp.tile([128, 512], F32, name="py", tag="py")
            nc.tensor.matmul(ps2[:, 0:512], wcrd[:, g * 128:(g + 1) * 128],
                             ct[:, t * 512:(t + 1) * 512], start=True, stop=True)
            for jj in range(2):
                nc.vector.tensor_scalar(
                    dest[2 * g + jj][64:128, t * 512:(t + 1) * 512],
                    ps2[jj * 64:(jj + 1) * 64, :], 5.0, -5.0, op0=MIN, op1=MAX)

        def v_pair(m):
            # V projection for key tiles 2m, 2m+1 with a single PSUM tile and
            # a single strided copy into the 65-stride vtile layout.
            t, i = divmod(2 * m, 4)
            pv = pyp.tile([128, 512], F32, name="py", tag="py")
            for half in range(2):
                n0 = t * 512 + (i + half) * 128
                for p in range(NP):
                    nc.tensor.matmul(
                        pv[:, half * 256:half * 256 + 256],
                        xt[p][:, n0:n0 + 128],
                        wv[:, p * 256:(p + 1) * 256],
                        start=(half == 0 and p == 0),
                        stop=(half == 1 and p == NP - 1))
            src = pv[:].rearrange("p (k h c) -> p k h c", k=2, h=4, c=64)
            dst = vtile[:, 2 * m * VW:(2 * m + 2) * VW].rearrange(
                "p (k h c) -> p k h c", k=2, h=4, c=65)[:, :, :, 0:64]
            nc.vector.tensor_copy(dst, src)

        # Pair order: head-group major. Block b covers (gg, qi); its 16
        # pairs are (h=2gg+hh, j) for hh in 0,1 and j in 0..7.
        order = [(gg, qi) for gg in range(2) for qi in range(NT)]

        def loc(g):
            b, s = divmod(g, 16)
            gg, qi = order[b]
            hh, j = divmod(s, 8)
            return qi, 2 * gg + hh, j

        def emit_dots(g):
            qi, h, j = loc(g)
            ps = psp.tile([128, 1024], F32, name="ps", tag="ps")
            ps_tiles[g] = ps
            k0, k1 = 2 * j, 2 * j + 1
            nc.tensor.matmul(
                ps[:, 0:512], kcat[h][:, k0 * 128:(k0 + 1) * 128],
                qcat[h][:, qi * 512:(qi + 1) * 512], start=True, stop=True)
            nc.tensor.matmul(
                ps[:, 512:1024], kcat[h][:, k1 * 128:(k1 + 1) * 128],
                qcat[h][:, qi * 512:(qi + 1) * 512], start=True, stop=True)

        def emit_av(g, pe):
            # Flipped AV: stationary = exp'd dots chunk [128k x 128q],
            # moving = V [128k x 65]; po accumulates [128q x 4*65].
            qi, h, j = loc(g)
            if j == 0:
                po_tiles[(qi, h)] = pop.tile([128, 4 * 65], F32,
                                             name="po", tag="po")
            po = po_tiles[(qi, h)]
            for half in range(2):
                kt = 2 * j + half
                for qs in range(4):
                    nc.tensor.matmul(
                        po[:, qs * 65:(qs + 1) * 65],
                        pe[:, half * 512 + qs * 128:half * 512 + (qs + 1) * 128],
                        vtile[:, kt * VW + h * 65:kt * VW + (h + 1) * 65],
                        start=(j == 0 and half == 0 and qs == 0),
                        stop=(j == NKT // 2 - 1 and half == 1 and qs == 3))

        def emit_pocopy(qi, h):
            poc = wk.tile([128, 4 * 65], BF16, name="poc", tag="poc")
            poc_tiles[(qi, h)] = poc
            nc.vector.tensor_copy(poc[:], po_tiles.pop((qi, h))[:])

        def emit_norm(qi, h):
            # Transpose the head's AV block back to [id, n] orientation and
            # normalize by the transposed denominator row.
            poc = poc_tiles.pop((qi, h))
            ot = pyp.tile([65, 512], BF16, name="py", tag="py")
            for qs in range(4):
                nc.tensor.matmul(ot[:, qs * 128:(qs + 1) * 128],
                                 poc[:, qs * 65:(qs + 1) * 65], idt[:],
                                 is_transpose=True,
                                 start=(qs == 0), stop=(qs == 3))
            rc = wk.tile([1, 512], F32, name="rc", tag="rc")
            rb = wk.tile([64, 512], F32, name="rb", tag="rb")
            nc.vector.reciprocal(rc[:], ot[64:65, :])
            nc.gpsimd.partition_broadcast(rb[:], rc[:])
            oj, rr = h // 2, (h % 2) * 64
            nc.vector.tensor_tensor(
                ocat[oj][rr:rr + 64, qi * 512:(qi + 1) * 512],
                ot[0:64, :], rb[:], op=MULT)

        def emit_phasec(qi, i, eng=None, q=None):
            n0 = qi * 512 + i * 128
            py = pyp.tile([128, 512], F32, name="py", tag="py")
            nc.tensor.matmul(py[:], ocat[0][:, n0:n0 + 128], wob[:, 0:512],
                             start=True, stop=False)
            nc.tensor.matmul(py[:], ocat[1][:, n0:n0 + 128], wob[:, 512:1024],
                             start=False, stop=True)
            st = wk.tile([128, 512], BF16, name="st", tag="st",
                         bufs=4)
            if eng is nc.scalar:
                nc.scalar.activation(st[:], py[:],
                                     mybir.ActivationFunctionType.Copy)
            else:
                nc.vector.tensor_copy(st[:], py[:])
            (q or nc.sync).dma_start(Y[n0:n0 + 128, :], st[:])

        def qproj_piece(t, g, p, pool=None, eng=None):
            # p 0..3: pixel accumulation chunk c=p; p 4: coord + clamps.
            if p == 0:
                if pool is None:
                    ps_tiles[("q", t, g)] = pyp.tile([128, 512], F32,
                                                     name="py", tag="py")
                else:
                    ps_tiles[("q", t, g)] = pool.tile([128, 512], F32,
                                                      name="ps", tag="ps")
            if p < 4:
                ps = ps_tiles[("q", t, g)]
                nc.tensor.matmul(
                    ps[:], wqp[:, p * 256 + g * 128:p * 256 + (g + 1) * 128],
                    xt[p][:, t * 512:(t + 1) * 512],
                    start=(p == 0), stop=(p == 3))
                if p == 3:
                    for jj in range(2):
                        nc.vector.tensor_scalar(
                            qcat[2 * g + jj][0:64, t * 512:(t + 1) * 512],
                            ps[jj * 64:(jj + 1) * 64, :], 5.0, -5.0,
                            op0=MIN, op1=MAX)
                    del ps_tiles[("q", t, g)]
            else:
                proj_crd(t, g, wqc, qcat, eng=eng)

        # ---- prologue ----
        proj_pix(0, 0, wkp, kcat, pool=psp)
        proj_crd(0, 0, wkc, kcat, eng=nc.vector)
        qproj_piece(0, 0, 0, pool=psp)
        qproj_piece(0, 0, 1)
        qproj_piece(0, 0, 2)
        qproj_piece(0, 0, 3)
        qproj_piece(0, 0, 4, eng=nc.vector)
        emit_dots(0)
        emit_dots(1)
        v_pair(0)

        # Fill schedule keyed by (block, slot). Block 0 carries the K/V
        # build for group 0; K for group 1 spreads over blocks 1-2; the next
        # block's Q projection occupies slots 8-12 of each block.
        fills = {}

        def add(b, s, fn):
            fills.setdefault((b, s), []).append(fn)

        add(0, 0, lambda: proj_pix(1, 0, wkp, kcat))
        add(0, 0, lambda: proj_crd(1, 0, wkc, kcat))
        add(0, 0, lambda: v_pair(1))
        add(0, 1, lambda: v_pair(2))
        add(0, 2, lambda: proj_pix(2, 0, wkp, kcat))
        add(0, 2, lambda: proj_crd(2, 0, wkc, kcat))
        add(0, 2, lambda: v_pair(3))
        add(0, 3, lambda: v_pair(4))
        add(0, 4, lambda: proj_pix(3, 0, wkp, kcat))
        add(0, 4, lambda: proj_crd(3, 0, wkc, kcat))
        add(0, 4, lambda: v_pair(5))
        add(0, 5, lambda: v_pair(6))
        add(0, 6, lambda: v_pair(7))
        for b, (tt, g) in enumerate([(1, 0), (2, 0), (3, 0), (0, 1),
                                     (1, 1), (2, 1), (3, 1)]):
            for p in range(5):
                add(b, 7 + p, lambda t=tt, g=g, p=p: qproj_piece(t, g, p))
        for b, t in [(1, 0), (1, 1), (2, 2), (2, 3)]:
            s = 2 if t % 2 == 0 else 5
            add(b, s, lambda t=t: proj_pix(t, 1, wkp, kcat))
            add(b, s + 1, lambda t=t: proj_crd(t, 1, wkc, kcat))

        NPAIR = NT * H_PER_CORE * (NKT // 2)
        for g in range(NPAIR):
            qi, h, j = loc(g)
            b, s = divmod(g, 16)
            pe = wk.tile([128, 1024], BF16, name="pe", tag="pe", bufs=3)
            nc.scalar.activation(pe[:], ps_tiles.pop(g)[:], EXP, scale=SCALE)
            if g % 8 == 0 and g >= 8:
                pqi, ph = loc(g - 8)[0], loc(g - 8)[1]
                emit_pocopy(pqi, ph)
            if g % 8 == 1 and g >= 9:
                pqi, ph = loc(g - 9)[0], loc(g - 9)[1]
                emit_norm(pqi, ph)
            if b == 0:
                # Block 0 is PE-oversubscribed and fills carry dots deps.
                for f in fills.get((b, s), []):
                    f()
                if g + 2 < NPAIR:
                    emit_dots(g + 2)
            else:
                if g + 2 < NPAIR:
                    emit_dots(g + 2)
                for f in fills.get((b, s), []):
                    f()
            emit_av(g, pe)
            # Output projection for query block qi once both head groups'
            # norms have landed: block (1, qi+1) = b index 5,6,7.
            if b >= 5 and s in (4, 6, 8, 10):
                emit_phasec(order[b][1] - 1, (s - 4) // 2)

        # ---- tail ----
        # The last head's norm is the only dependency of the final output
        # projections' ocat[1] reads; the ocat[0] halves are emitted first so
        # they overlap the norm chain. py tiles come from both PSUM pools so
        # four can be in flight.
        emit_pocopy(NT - 1, H_PER_CORE - 1)
        tail_py = {}
        n_base = (NT - 1) * 512

        def tail_j0(i):
            n0 = n_base + i * 128
            py = (pyp if i % 2 == 0 else pop).tile(
                [128, 512], F32, name="py" if i % 2 == 0 else "po",
                tag="py" if i % 2 == 0 else "po")
            tail_py[i] = py
            nc.tensor.matmul(py[:], ocat[0][:, n0:n0 + 128], wob[:, 0:512],
                             start=True, stop=False)

        def tail_j1(i):
            n0 = n_base + i * 128
            py = tail_py[i]
            nc.tensor.matmul(py[:], ocat[1][:, n0:n0 + 128], wob[:, 512:1024],
                             start=False, stop=True)
            st = wk.tile([128, 512], BF16, name="st", tag="st", bufs=4)
            if i % 2 == 0:
                nc.scalar.activation(st[:], py[:],
                                     mybir.ActivationFunctionType.Copy)
            else:
                nc.vector.tensor_copy(st[:], py[:])
            (nc.sync if i % 2 == 0 else nc.scalar).dma_start(
                Y[n0:n0 + 128, :], st[:])

        tail_j0(0)
        tail_j0(1)
        emit_norm(NT - 1, H_PER_CORE - 1)
        tail_j1(0)
        tail_j1(1)
        tail_j0(2)
        tail_j1(2)
        tail_j0(3)
        tail_j1(3)
    nc.compile()
    return nc


def _get_nc():
    global _NC
    if _NC is None:
        _NC = _build()
    return _NC


def _pack(w, nblk, blk):
    w = np.asarray(w, dtype=np.float32)
    return np.ascontiguousarray(
        w.reshape(nblk, 128, blk).transpose(1, 0, 2).reshape(128, nblk * blk))


def kernel(pixels, coords, mask, W_qkv, W_qkc, W_out, b_out):
    global LAST_EXEC_NS
    pixels = np.asarray(pixels, dtype=np.float32)
    coords = np.asarray(coords, dtype=np.float32)
    W_qkv = np.asarray(W_qkv, dtype=np.float32)
    W_qkc = np.asarray(W_qkc, dtype=np.float32)
    W_out = np.asarray(W_out, dtype=np.float32)
    b_out = np.asarray(b_out, dtype=np.float32)

    nc = _get_nc()

    XT = [np.ascontiguousarray(pixels[b].T) for b in range(B)]
    CT = [np.ascontiguousarray(coords[b].T) for b in range(B)]
    ident = np.eye(128, dtype=ml_dtypes.bfloat16)

    in_maps = []
    for c in range(8):
        b = c // 2
        h0 = (c % 2) * H_PER_CORE * DH     # 0 or 256: col offset within split
        wkp = _pack(W_qkv[:, ID + h0:ID + h0 + 256], 4, 256)
        wqp = _pack(W_qkv[:, h0:h0 + 256], 4, 256)
        wv = _pack(W_qkv[:, 2 * ID + h0:2 * ID + h0 + 256], 4, 256)
        wkc = np.zeros((128, 256), np.float32)
        wkc[:CD] = W_qkc[:, ID + h0:ID + h0 + 256]
        wqc = np.zeros((128, 256), np.float32)
        wqc[:CD] = W_qkc[:, h0:h0 + 256]
        wob = _pack(W_out[h0:h0 + 256, :], 2, 512)
        w1 = np.hstack([wkp, wkc, wqp, wqc]).astype(ml_dtypes.bfloat16)
        w2 = np.hstack([wv, ident.astype(np.float32), wob]).astype(
            ml_dtypes.bfloat16)
        in_maps.append({
            "XT": XT[b].astype(ml_dtypes.bfloat16),
            "CT": CT[b].astype(ml_dtypes.bfloat16),
            "W1": w1,
            "W2": w2,
        })

    res = run_bass_kernel_spmd(nc, in_maps, core_ids=list(range(8)))
    LAST_EXEC_NS = getattr(res, "exec_time_ns", None)

    out = np.empty((B, N, OUT_D), np.float32)
    for b in range(B):
        out[b] = (res.results[2 * b]["Y"].astype(np.float32) +
                  res.results[2 * b + 1]["Y"].astype(np.float32))
    out += b_out[None, None, :]
    return tuple(np.split(out, [1024], axis=1))


# revision 6
# speedup vs baseline: 1.2688x; 1.0085x over previous
import numpy as np
import ml_dtypes

import concourse.bacc as bacc
import concourse.tile as tile
import concourse.mybir as mybir
from concourse.bass_utils import run_bass_kernel_spmd

F32 = mybir.dt.float32
F32R = mybir.dt.float32r
BF16 = mybir.dt.bfloat16

B = 4
N = 2048
PD = 512
CD = 128
ID = 512
OUT_D = 512
H_PER_CORE = 4
DH = 64
SCALE = 0.125          # dim_head ** -0.5
NT = 4                 # n chunks of 512 (query blocks)
NP = 4                 # pd chunks of 128
NKT = 16               # key tiles of 128
VW = 65 * H_PER_CORE   # vtile columns per key tile: 4 x (64 V | 1 one)

_NC = None
LAST_EXEC_NS = None


def _build():
    nc = bacc.Bacc("TRN2", target_bir_lowering=False, debug=False, num_devices=8)
    XT = nc.declare_dram_parameter("XT", [PD, N], BF16, isOutput=False)
    CT = nc.declare_dram_parameter("CT", [CD, N], BF16, isOutput=False)
    W1 = nc.declare_dram_parameter("W1", [128, 2560], BF16, isOutput=False)
    W2 = nc.declare_dram_parameter("W2", [128, 2176], BF16, isOutput=False)
    Y = nc.declare_dram_parameter("Y", [N, OUT_D], BF16, isOutput=True)

    MIN = mybir.AluOpType.min
    MAX = mybir.AluOpType.max
    MULT = mybir.AluOpType.mult
    EXP = mybir.ActivationFunctionType.Exp

    with tile.TileContext(nc) as tc, \
         tc.tile_pool(name="persist", bufs=1) as pp, \
         tc.tile_pool(name="work", bufs=2) as wk, \
         tc.tile_pool(name="pb_s", bufs=2, space="PSUM") as psp, \
         tc.tile_pool(name="pb_o", bufs=2, space="PSUM") as pop, \
         tc.tile_pool(name="pb_y", bufs=2, space="PSUM") as pyp:
        x4 = pp.tile([128, NP * N], BF16, name="x4", tag="x4")
        xt = [x4[:, p * N:(p + 1) * N] for p in range(NP)]
        ct = pp.tile([128, N], BF16, name="ct", tag="ct")
        w1 = pp.tile([128, 2560], BF16, name="w1", tag="w1")
        wkp = w1[:, 0:1024]
        wkc = w1[:, 1024:1280]
        wqp = w1[:, 1280:2304]
        wqc = w1[:, 2304:2560]
        w2 = pp.tile([128, 2176], BF16, name="w2", tag="w2")
        wv = w2[:, 0:1024]
        idt = w2[:, 1024:1152]
        wob = w2[:, 1152:2176]
        qcat = [pp.tile([128, N], F32R, name=f"qcat{h}", tag=f"qcat{h}")
                for h in range(H_PER_CORE)]
        kcat = [pp.tile([128, N], F32R, name=f"kcat{h}", tag=f"kcat{h}")
                for h in range(H_PER_CORE)]
        vtile = pp.tile([128, NKT * VW], BF16, name="vtile", tag="vtile")
        ocat = [pp.tile([128, N], BF16, name=f"ocat{j}", tag=f"ocat{j}")
                for j in range(2)]
        zwarm = pp.tile([128, 512], BF16, name="zwarm", tag="zwarm")

        # PE warmup is enqueued before any DMA so the p-state ramp starts
        # immediately: the zwarm memset goes on Pool ahead of its DMA issues.
        nc.gpsimd.memset(zwarm[:], 0.0)
        pwarm = pyp.tile([128, 512], F32, name="py", tag="py")
        for _ in range(8):
            nc.tensor.matmul(pwarm[:], zwarm[:, 0:128], zwarm[:],
                             start=True, stop=True)

        # Input DMAs over three queues (SP / Pool / ACT doorbells), ordered
        # so the K/Q projection prerequisites for the t=0 block land first.
        # The ACT-queue DMAs are configured before the first Activation and
        # only carry tensors that aren't needed until well into the run.
        nc.gpsimd.memset(vtile[:], 1.0)
        x4v = x4[:].rearrange("p (c n) -> p c n", c=NP, n=N)
        xtv = XT[:].rearrange("(c r) n -> r c n", c=NP, r=128)
        nc.sync.dma_start(w1[:], W1[:])
        nc.scalar.dma_start(ct[:, 0:512], CT[:, 0:512])
        nc.sync.dma_start(x4v[:, :, 0:512], xtv[:, :, 0:512])
        nc.scalar.dma_start(w2[:], W2[:])
        nc.sync.dma_start(x4v[:, :, 512:1024], xtv[:, :, 512:1024])
        nc.scalar.dma_start(ct[:, 512:2048], CT[:, 512:2048])
        nc.sync.dma_start(x4v[:, :, 1024:1536], xtv[:, :, 1024:1536])
        nc.sync.dma_start(x4v[:, :, 1536:2048], xtv[:, :, 1536:2048])

        ps_tiles = {}
        po_tiles = {}
        poc_tiles = {}

        def proj_pix(t, g, wpix, dest, pool=None):
            # pixel half of the q/k projection for block t, head group g:
            # clamped rows 0:64 of heads 2g, 2g+1. Clamps on DVE.
            if pool is None:
                ps = pyp.tile([128, 512], F32, name="py", tag="py")
            else:
                ps = pool.tile([128, 512], F32, name="ps", tag="ps")
            for c in range(NP):
                nc.tensor.matmul(
                    ps[:], wpix[:, c * 256 + g * 128:c * 256 + (g + 1) * 128],
                    xt[c][:, t * 512:(t + 1) * 512],
                    start=(c == 0), stop=(c == NP - 1))
            for jj in range(2):
                nc.vector.tensor_scalar(
                    dest[2 * g + jj][0:64, t * 512:(t + 1) * 512],
                    ps[jj * 64:(jj + 1) * 64, :], 5.0, -5.0, op0=MIN, op1=MAX)

        def proj_crd(t, g, wcrd, dest, eng=None):
            # coord half: clamped rows 64:128. Clamps on Pool by default.
            ps2 = pyp.tile([128, 512], F32, name="py", tag="py")
            nc.tensor.matmul(ps2[:, 0:512], wcrd[:, g * 128:(g + 1) * 128],
                             ct[:, t * 512:(t + 1) * 512], start=True, stop=True)
            for jj in range(2):
                nc.vector.tensor_scalar(
                    dest[2 * g + jj][64:128, t * 512:(t + 1) * 512],
                    ps2[jj * 64:(jj + 1) * 64, :], 5.0, -5.0, op0=MIN, op1=MAX)

        def v_pair(m):
            # V projection for key tiles 2m, 2m+1 with a single PSUM tile and
            # a single strided copy into the 65-stride vtile layout.
            t, i = divmod(2 * m, 4)
            pv = pyp.tile([128, 512], F32, name="py", tag="py")
            for half in range(2):
                n0 = t * 512 + (i + half) * 128
                for p in range(NP):
                    nc.tensor.matmul(
                        pv[:, half * 256:half * 256 + 256],
                        xt[p][:, n0:n0 + 128],
                        wv[:, p * 256:(p + 1) * 256],
                        start=(half == 0 and p == 0),
                        stop=(half == 1 and p == NP - 1))
            src = pv[:].rearrange("p (k h c) -> p k h c", k=2, h=4, c=64)
            dst = vtile[:, 2 * m * VW:(2 * m + 2) * VW].rearrange(
                "p (k h c) -> p k h c", k=2, h=4, c=65)[:, :, :, 0:64]
            nc.vector.tensor_copy(dst, src)

        # Pair order: head-group major. Block b covers (gg, qi); its 16
        # pairs are (h=2gg+hh, j) for hh in 0,1 and j in 0..7.
        order = [(gg, qi) for gg in range(2) for qi in range(NT)]

        def loc(g):
            b, s = divmod(g, 16)
            gg, qi = order[b]
            if b == 0:
                hh, j = s % 2, s // 2
            else:
                hh, j = divmod(s, 8)
            return qi, 2 * gg + hh, j

        def emit_dots(g):
            qi, h, j = loc(g)
            ps = psp.tile([128, 1024], F32, name="ps", tag="ps")
            ps_tiles[g] = ps
            k0, k1 = 2 * j, 2 * j + 1
            nc.tensor.matmul(
                ps[:, 0:512], kcat[h][:, k0 * 128:(k0 + 1) * 128],
                qcat[h][:, qi * 512:(qi + 1) * 512], start=True, stop=True)
            nc.tensor.matmul(
                ps[:, 512:1024], kcat[h][:, k1 * 128:(k1 + 1) * 128],
                qcat[h][:, qi * 512:(qi + 1) * 512], start=True, stop=True)

        def emit_av(g, pe):
            # Flipped AV: stationary = exp'd dots chunk [128k x 128q],
            # moving = V [128k x 65]; po accumulates [128q x 4*65].
            qi, h, j = loc(g)
            if j == 0:
                po_tiles[(qi, h)] = pop.tile([128, 4 * 65], F32,
                                             name="po", tag="po")
            po = po_tiles[(qi, h)]
            for half in range(2):
                kt = 2 * j + half
                for qs in range(4):
                    nc.tensor.matmul(
                        po[:, qs * 65:(qs + 1) * 65],
                        pe[:, half * 512 + qs * 128:half * 512 + (qs + 1) * 128],
                        vtile[:, kt * VW + h * 65:kt * VW + (h + 1) * 65],
                        start=(j == 0 and half == 0 and qs == 0),
                        stop=(j == NKT // 2 - 1 and half == 1 and qs == 3))

        def emit_pocopy(qi, h):
            poc = wk.tile([128, 4 * 65], BF16, name="poc", tag="poc")
            poc_tiles[(qi, h)] = poc
            nc.vector.tensor_copy(poc[:], po_tiles.pop((qi, h))[:])

        def emit_norm(qi, h):
            # Transpose the head's AV block back to [id, n] orientation and
            # normalize by the transposed denominator row.
            poc = poc_tiles.pop((qi, h))
            ot = pyp.tile([65, 512], BF16, name="py", tag="py")
            for qs in range(4):
                nc.tensor.matmul(ot[:, qs * 128:(qs + 1) * 128],
                                 poc[:, qs * 65:(qs + 1) * 65], idt[:],
                                 is_transpose=True,
                                 start=(qs == 0), stop=(qs == 3))
            rc = wk.tile([1, 512], F32, name="rc", tag="rc")
            rb = wk.tile([64, 512], F32, name="rb", tag="rb")
            nc.vector.reciprocal(rc[:], ot[64:65, :])
            nc.gpsimd.partition_broadcast(rb[:], rc[:])
            oj, rr = h // 2, (h % 2) * 64
            nc.vector.tensor_tensor(
                ocat[oj][rr:rr + 64, qi * 512:(qi + 1) * 512],
                ot[0:64, :], rb[:], op=MULT)

        phasec_py = {}

        def emit_phasec_j0(qi, i):
            n0 = qi * 512 + i * 128
            py = pyp.tile([128, 512], F32, name="py", tag="py")
            phasec_py[(qi, i)] = py
            nc.tensor.matmul(py[:], ocat[0][:, n0:n0 + 128], wob[:, 0:512],
                             start=True, stop=False)

        def emit_phasec_j1(qi, i, eng=None, q=None):
            n0 = qi * 512 + i * 128
            py = phasec_py.pop((qi, i))
            nc.tensor.matmul(py[:], ocat[1][:, n0:n0 + 128], wob[:, 512:1024],
                             start=False, stop=True)
            st = wk.tile([128, 512], BF16, name="st", tag="st",
                         bufs=4)
            if eng is nc.scalar:
                nc.scalar.activation(st[:], py[:],
                                     mybir.ActivationFunctionType.Copy)
            else:
                nc.vector.tensor_copy(st[:], py[:])
            (q or nc.sync).dma_start(Y[n0:n0 + 128, :], st[:])

        def qproj_piece(t, g, p, pool=None, eng=None):
            # p 0..3: pixel accumulation chunk c=p; p 4: coord + clamps.
            if p == 0:
                if pool is None:
                    ps_tiles[("q", t, g)] = pyp.tile([128, 512], F32,
                                                     name="py", tag="py")
                else:
                    ps_tiles[("q", t, g)] = pool.tile([128, 512], F32,
                                                      name="ps", tag="ps")
            if p < 4:
                ps = ps_tiles[("q", t, g)]
                nc.tensor.matmul(
                    ps[:], wqp[:, p * 256 + g * 128:p * 256 + (g + 1) * 128],
                    xt[p][:, t * 512:(t + 1) * 512],
                    start=(p == 0), stop=(p == 3))
                if p == 3:
                    for jj in range(2):
                        nc.vector.tensor_scalar(
                            qcat[2 * g + jj][0:64, t * 512:(t + 1) * 512],
                            ps[jj * 64:(jj + 1) * 64, :], 5.0, -5.0,
                            op0=MIN, op1=MAX)
                    del ps_tiles[("q", t, g)]
            else:
                proj_crd(t, g, wqc, qcat, eng=eng)

        # ---- prologue ----
        # t0 projections with clamps ordered head-0-first so dots(0) waits
        # on four clamps instead of eight.
        kxp = psp.tile([128, 512], F32, name="ps", tag="ps")
        for c in range(NP):
            nc.tensor.matmul(kxp[:], wkp[:, c * 256:c * 256 + 128],
                             xt[c][:, 0:512], start=(c == 0), stop=(c == 3))
        qxp = psp.tile([128, 512], F32, name="ps", tag="ps")
        for c in range(NP):
            nc.tensor.matmul(qxp[:], wqp[:, c * 256:c * 256 + 128],
                             xt[c][:, 0:512], start=(c == 0), stop=(c == 3))
        kxc = pyp.tile([128, 512], F32, name="py", tag="py")
        nc.tensor.matmul(kxc[:], wkc[:, 0:128], ct[:, 0:512],
                         start=True, stop=True)
        qxc = pyp.tile([128, 512], F32, name="py", tag="py")
        nc.tensor.matmul(qxc[:], wqc[:, 0:128], ct[:, 0:512],
                         start=True, stop=True)
        for jj in range(2):
            for src, dest, rows in ((kxp, kcat, 0), (qxp, qcat, 0),
                                    (kxc, kcat, 64), (qxc, qcat, 64)):
                nc.vector.tensor_scalar(
                    dest[jj][rows:rows + 64, 0:512],
                    src[jj * 64:(jj + 1) * 64, :], 5.0, -5.0,
                    op0=MIN, op1=MAX)
        emit_dots(0)
        emit_dots(1)
        v_pair(0)

        # Fill schedule keyed by (block, slot). Block 0 carries the K/V
        # build for group 0; K for group 1 spreads over blocks 1-2; the next
        # block's Q projection occupies slots 8-12 of each block.
        fills = {}

        def add(b, s, fn):
            fills.setdefault((b, s), []).append(fn)

        add(0, 0, lambda: proj_pix(1, 0, wkp, kcat))
        add(0, 1, lambda: proj_crd(1, 0, wkc, kcat))
        add(0, 2, lambda: v_pair(1))
        add(0, 3, lambda: v_pair(2))
        add(0, 4, lambda: proj_pix(2, 0, wkp, kcat))
        add(0, 5, lambda: proj_crd(2, 0, wkc, kcat))
        add(0, 6, lambda: v_pair(3))
        add(0, 7, lambda: v_pair(4))
        add(0, 8, lambda: proj_pix(3, 0, wkp, kcat))
        add(0, 9, lambda: proj_crd(3, 0, wkc, kcat))
        add(0, 10, lambda: v_pair(5))
        add(0, 11, lambda: v_pair(6))
        add(0, 12, lambda: v_pair(7))
        for b, (tt, g) in enumerate([(1, 0), (2, 0), (3, 0), (0, 1),
                                     (1, 1), (2, 1), (3, 1)]):
            for p in range(5):
                add(b, 6 + p, lambda t=tt, g=g, p=p: qproj_piece(t, g, p))
        for b, t in [(1, 0), (1, 1), (2, 2), (2, 3)]:
            s = 2 if t % 2 == 0 else 5
            add(b, s, lambda t=t: proj_pix(t, 1, wkp, kcat))
            add(b, s + 1, lambda t=t: proj_crd(t, 1, wkc, kcat))

        NPAIR = NT * H_PER_CORE * (NKT // 2)
        for g in range(NPAIR):
            qi, h, j = loc(g)
            b, s = divmod(g, 16)
            pe = wk.tile([128, 1024], BF16, name="pe", tag="pe", bufs=3)
            nc.scalar.activation(pe[:], ps_tiles.pop(g)[:], EXP, scale=SCALE)
            if g % 8 == 0 and g >= 16:
                pqi, ph = loc(g - 8)[0], loc(g - 8)[1]
                emit_pocopy(pqi, ph)
            if g % 8 == 1 and g >= 17:
                pqi, ph = loc(g - 9)[0], loc(g - 9)[1]
                emit_norm(pqi, ph)
            if g == 18:
                emit_pocopy(0, 1)
            if g == 19:
                emit_norm(0, 1)
            if b == 0:
                # Block 0 is PE-oversubscribed and fills carry dots deps.
                for f in fills.get((b, s), []):
                    f()
                if g + 2 < NPAIR:
                    emit_dots(g + 2)
            else:
                if g + 2 < NPAIR:
                    emit_dots(g + 2)
                for f in fills.get((b, s), []):
                    f()
            emit_av(g, pe)
            # Output projection for query block qi once both head groups'
            # norms have landed: block (1, qi+1) = b index 5,6,7.
            if b >= 5 and s in (2, 4, 12, 14):
                emit_phasec_j0(order[b][1] - 1, (2, 4, 12, 14).index(s))
            if b >= 5 and s in (3, 5, 13, 15):
                emit_phasec_j1(order[b][1] - 1, (3, 5, 13, 15).index(s))

        # ---- tail ----
        # The last head's norm is the only dependency of the final output
        # projections' ocat[1] reads; the ocat[0] halves are emitted first so
        # they overlap the norm chain. py tiles come from both PSUM pools so
        # four can be in flight.
        emit_pocopy(NT - 1, H_PER_CORE - 1)
        tail_py = {}
        n_base = (NT - 1) * 512

        def tail_j0(i):
            n0 = n_base + i * 128
            py = (pyp if i % 2 == 0 else pop).tile(
                [128, 512], F32, name="py" if i % 2 == 0 else "po",
                tag="py" if i % 2 == 0 else "po")
            tail_py[i] = py
            nc.tensor.matmul(py[:], ocat[0][:, n0:n0 + 128], wob[:, 0:512],
                             start=True, stop=False)

        def tail_j1(i):
            n0 = n_base + i * 128
            py = tail_py[i]
            nc.tensor.matmul(py[:], ocat[1][:, n0:n0 + 128], wob[:, 512:1024],
                             start=False, stop=True)
            st = wk.tile([128, 512], BF16, name="st", tag="st", bufs=4)
            if i % 2 == 0:
                nc.scalar.activation(st[:], py[:],
                                     mybir.ActivationFunctionType.Copy)
            else:
                nc.vector.tensor_copy(st[:], py[:])
            (nc.sync if i % 2 == 0 else nc.scalar).dma_start(
                Y[n0:n0 + 128, :], st[:])

        tail_j0(0)
        tail_j0(1)
        emit_norm(NT - 1, H_PER_CORE - 1)
        tail_j1(0)
        tail_j1(1)
        tail_j0(2)
        tail_j1(2)
        tail_j0(3)
        tail_j1(3)
    nc.compile()
    return nc


def _get_nc():
    global _NC
    if _NC is None:
        _NC = _build()
    return _NC


def _pack(w, nblk, blk):
    w = np.asarray(w, dtype=np.float32)
    return np.ascontiguousarray(
        w.reshape(nblk, 128, blk).transpose(1, 0, 2).reshape(128, nblk * blk))


def kernel(pixels, coords, mask, W_qkv, W_qkc, W_out, b_out):
    global LAST_EXEC_NS
    pixels = np.asarray(pixels, dtype=np.float32)
    coords = np.asarray(coords, dtype=np.float32)
    W_qkv = np.asarray(W_qkv, dtype=np.float32)
    W_qkc = np.asarray(W_qkc, dtype=np.float32)
    W_out = np.asarray(W_out, dtype=np.float32)
    b_out = np.asarray(b_out, dtype=np.float32)

    nc = _get_nc()

    XT = [np.ascontiguousarray(pixels[b].T) for b in range(B)]
    CT = [np.ascontiguousarray(coords[b].T) for b in range(B)]
    ident = np.eye(128, dtype=ml_dtypes.bfloat16)

    in_maps = []
    for c in range(8):
        b = c // 2
        h0 = (c % 2) * H_PER_CORE * DH     # 0 or 256: col offset within split
        wkp = _pack(W_qkv[:, ID + h0:ID + h0 + 256], 4, 256)
        wqp = _pack(W_qkv[:, h0:h0 + 256], 4, 256)
        wv = _pack(W_qkv[:, 2 * ID + h0:2 * ID + h0 + 256], 4, 256)
        wkc = np.zeros((128, 256), np.float32)
        wkc[:CD] = W_qkc[:, ID + h0:ID + h0 + 256]
        wqc = np.zeros((128, 256), np.float32)
        wqc[:CD] = W_qkc[:, h0:h0 + 256]
        wob = _pack(W_out[h0:h0 + 256, :], 2, 512)
        w1 = np.hstack([wkp, wkc, wqp, wqc]).astype(ml_dtypes.bfloat16)
        w2 = np.hstack([wv, ident.astype(np.float32), wob]).astype(
            ml_dtypes.bfloat16)
        in_maps.append({
            "XT": XT[b].astype(ml_dtypes.bfloat16),
            "CT": CT[b].astype(ml_dtypes.bfloat16),
            "W1": w1,
            "W2": w2,
        })

    res = run_bass_kernel_spmd(nc, in_maps, core_ids=list(range(8)))
    LAST_EXEC_NS = getattr(res, "exec_time_ns", None)

    out = np.empty((B, N, OUT_D), np.float32)
    for b in range(B):
        out[b] = (res.results[2 * b]["Y"].astype(np.float32) +
                  res.results[2 * b + 1]["Y"].astype(np.float32))
    out += b_out[None, None, :]
    return tuple(np.split(out, [1024], axis=1))
